# revision 25
# baseline (speedup 1.0000x reference)
"""Bidirectional Mamba TRN2 kernel (8 NeuronCores, SPMD) — v2.

Sharding: core c owns batch b = c//4 and dtiles (c%4)*3 + {0,1,2} (3 tiles of
128 d_inner channels), BOTH scan directions. x_proj partials are AllReduced
per-batch (groups {0..3},{4..7}); out_proj partials summed on the host.

v2 structure (vs v1): full-L phase 2 (one 4096-long scan per (n,j,dir); no
quartering, no m_spill, no scan-state chaining), u/z kept on-chip or in small
bf16 spills, B/C rows broadcast by DMA from a bf16 DRAM staging row
(replicated-partition read) instead of GPSIMD partition_broadcast, the
16-state output accumulation m runs on GPSIMD in fp32 (DVE does only
d1/scan/hc), D-terms via ScalarE scale-copy, and every matmul is bf16.
"""
import numpy as np
from contextlib import ExitStack

import ml_dtypes
import concourse.bass as bass
import concourse.bacc as bacc
import concourse.tile as tile
from concourse import mybir, library_config
from concourse.bass_utils import run_bass_kernel_spmd

B, L, D = 2, 4096, 768
DI, DS, DTR, KC = 1536, 16, 48, 4
NCORES = 8
NPAIR = 3                 # dtiles per core
P = 128
NKT = D // P              # 6 K-tiles for in_proj
LC = 512                  # matmul free-dim chunk
NLC = L // LC             # 8
E = DTR + 2 * DS          # 80

f32 = mybir.dt.float32
bf16 = mybir.dt.bfloat16
ALU = mybir.AluOpType
AF = mybir.ActivationFunctionType
bfnp = ml_dtypes.bfloat16


def build_module():
    nc = bacc.Bacc("TRN2", target_bir_lowering=False, debug=False,
                   num_devices=NCORES)

    # ---- external inputs (per-core data; same tensor names on all cores) ----
    hT = nc.dram_tensor("hT", [D, L], bf16, kind="ExternalInput")
    w_in = nc.dram_tensor("w_in", [D, 2 * NPAIR * P], bf16, kind="ExternalInput")
    convw = nc.dram_tensor("convw", [2, NPAIR, P, KC], f32, kind="ExternalInput")
    convb = nc.dram_tensor("convb", [2, NPAIR, P], f32, kind="ExternalInput")
    w_xp = nc.dram_tensor("w_xp", [2, NPAIR * P, E], bf16, kind="ExternalInput")
    w_dt = nc.dram_tensor("w_dt", [2, NPAIR, DTR, P], bf16, kind="ExternalInput")
    dt_bias = nc.dram_tensor("dt_bias", [2, NPAIR, P], f32, kind="ExternalInput")
    Acol = nc.dram_tensor("Acol", [2, NPAIR, P, DS], f32, kind="ExternalInput")
    Dvec = nc.dram_tensor("Dvec", [2, NPAIR, P], f32, kind="ExternalInput")
    w_out = nc.dram_tensor("w_out", [NPAIR, P, D], bf16, kind="ExternalInput")
    out_part = nc.dram_tensor("out_part", [D, L], f32, kind="ExternalOutput")

    # ---- internal DRAM ----
    z16_dram = nc.dram_tensor("z16_dram", [NPAIR, P, L], bf16)
    u16_dram = nc.dram_tensor("u16_dram", [2, NPAIR, P, L], bf16)
    cc_in = nc.dram_tensor("cc_in", [2, E, L], f32)
    cc_out = nc.dram_tensor("cc_out", [2, E, L], f32)
    rows_dram = nc.dram_tensor("rows_dram", [2, 2 * DS, L], bf16)

    with tile.TileContext(nc) as tc, ExitStack() as top:
        wp = top.enter_context(tc.tile_pool(name="weights", bufs=1))
        pm = top.enter_context(tc.tile_pool(name="persist", bufs=1))

        nc.gpsimd.load_library(library_config.proxy)

        # ---- long-lived weights ----
        w_dt_sb = wp.tile([DTR, 2, NPAIR, P], bf16, tag="w_dt", name="w_dt")
        nc.sync.dma_start(w_dt_sb[:], w_dt.ap().rearrange("d j r p -> r d j p"))
        dtb_sb = wp.tile([P, 2, NPAIR], f32, tag="dtb", name="dtb")
        nc.sync.dma_start(dtb_sb[:], dt_bias.ap().rearrange("d j p -> p d j"))
        Acol_sb = wp.tile([P, 2, NPAIR, DS], f32, tag="Acol", name="Acol")
        nc.sync.dma_start(Acol_sb[:], Acol.ap().rearrange("d j p n -> p d j n"))
        D_sb = wp.tile([P, 2, NPAIR], f32, tag="Dsb", name="Dsb")
        nc.sync.dma_start(D_sb[:], Dvec.ap().rearrange("d j p -> p d j"))
        w_out_sb = wp.tile([P, NPAIR, D], bf16, tag="w_out", name="w_out")
        nc.sync.dma_start(w_out_sb[:], w_out.ap().rearrange("j p c -> p j c"))

        # persistent output accumulators, one per direction (bf16, DVE 2x)
        macc = {(dr, j): pm.tile([P, L], bf16, tag=f"m_{dr}_{j}",
                                 name=f"m_{dr}_{j}")
                for dr in range(2) for j in range(NPAIR)}

        # =========== Phase 1: in_proj, z-silu, conv, u, dbl partials ==========
        with ExitStack() as p1:
            wp1 = p1.enter_context(tc.tile_pool(name="p1w", bufs=1))
            x16p = p1.enter_context(tc.tile_pool(name="x16", bufs=1))
            tp1 = p1.enter_context(tc.tile_pool(name="p1tmp", bufs=2))
            bp1 = p1.enter_context(tc.tile_pool(name="p1big", bufs=1))
            dblp = p1.enter_context(tc.tile_pool(name="dblsb", bufs=1))

            w_in_sb = wp1.tile([P, NKT, 2 * NPAIR * P], bf16, tag="w_in", name="w_in")
            nc.sync.dma_start(w_in_sb[:],
                              w_in.ap().rearrange("(kt p) c -> p kt c", p=P))
            convw_sb = wp1.tile([P, 2, NPAIR, KC], f32, tag="convw", name="convw")
            nc.sync.dma_start(convw_sb[:],
                              convw.ap().rearrange("d j p k -> p d j k"))
            convb_sb = wp1.tile([P, 2, NPAIR], f32, tag="convb", name="convb")
            nc.sync.dma_start(convb_sb[:],
                              convb.ap().rearrange("d j p -> p d j"))
            w_xp_sb = wp1.tile([P, 2, NPAIR, E], bf16, tag="w_xp", name="w_xp")
            nc.sync.dma_start(w_xp_sb[:],
                              w_xp.ap().rearrange("d (j p) e -> p d j e", p=P))

            x16 = [x16p.tile([P, L], bf16, tag=f"x16_{j}", name=f"x16_{j}")
                   for j in range(NPAIR)]
            u16 = {(dr, j): x16p.tile([P, L], bf16, tag=f"u16_{dr}_{j}",
                                      name=f"u16_{dr}_{j}")
                   for dr in range(2) for j in range(NPAIR)}

            def conv_u(dr, j):
                """Depthwise causal conv + silu -> u16[dr, j]; init m D-term."""
                acc = bp1.tile([P, L], bf16, tag="cacc", name="cacc")
                w = lambda k: convw_sb[:, dr, j, k:k + 1]
                if dr == 0:   # taps k read x[t-3+k]
                    nc.vector.tensor_scalar(acc[:], x16[j][:], w(3),
                                            convb_sb[:, dr, j:j + 1],
                                            op0=ALU.mult, op1=ALU.add)
                    for k in range(3):
                        sh = 3 - k
                        nc.vector.scalar_tensor_tensor(
                            acc[:, sh:L], x16[j][:, 0:L - sh], w(k),
                            acc[:, sh:L], op0=ALU.mult, op1=ALU.add)
                else:         # host-reversed taps jj read x[t+jj]
                    nc.vector.tensor_scalar(acc[:], x16[j][:], w(0),
                                            convb_sb[:, dr, j:j + 1],
                                            op0=ALU.mult, op1=ALU.add)
                    for jj in range(1, 4):
                        nc.vector.scalar_tensor_tensor(
                            acc[:, 0:L - jj], x16[j][:, jj:L], w(jj),
                            acc[:, 0:L - jj], op0=ALU.mult, op1=ALU.add)
                sg = bp1.tile([P, L], f32, tag="usg", name="usg")
                nc.scalar.activation(sg[:], acc[:], AF.Sigmoid)
                u = u16[dr, j]
                nc.vector.tensor_tensor(u[:], acc[:], sg[:], op=ALU.mult)
                nc.sync.dma_start(u16_dram.ap()[dr, j], u[:])
                nc.scalar.activation(macc[dr, j][:], u[:], AF.Copy,
                                     scale=D_sb[:, dr, j:j + 1])

            def dbl_cc(dr, dblps):
                """x_proj partial matmuls for one dir, then its AllReduce."""
                for lc in range(NLC):
                    cols = slice(lc * LC, (lc + 1) * LC)
                    dps = dblps.tile([E, LC], f32, tag="dblps", name="dblps")
                    for j in range(NPAIR):
                        nc.tensor.matmul(
                            dps[:], w_xp_sb[:, dr, j, :],
                            u16[dr, j][:, cols],
                            start=(j == 0), stop=(j == NPAIR - 1))
                    dsb = tp1.tile([E, LC], f32, tag="dsb", name="dsb")
                    nc.scalar.copy(dsb[:], dps[:])
                    nc.sync.dma_start(cc_in.ap()[dr, :, cols], dsb[:])
                nc.gpsimd.collective_compute(
                    "AllReduce", ALU.add,
                    replica_groups=[[0, 1, 2, 3], [4, 5, 6, 7]],
                    ins=[cc_in.ap()[dr]], outs=[cc_out.ap()[dr]])

            # --- in_proj j-major (bf16 matmuls), conv_a interleaved per j ---
            with ExitStack() as s1:
                xzps = s1.enter_context(
                    tc.tile_pool(name="xzps", bufs=1, space="PSUM"))
                dblps = s1.enter_context(
                    tc.tile_pool(name="dblps", bufs=2, space="PSUM"))
                acc_ps = [xzps.tile([P, LC], f32, tag=f"xz{s}", name=f"xz{s}")
                          for s in range(2)]
                for j in range(NPAIR):
                    for lc in range(NLC):
                        cols = slice(lc * LC, (lc + 1) * LC)
                        for kt in range(NKT):
                            rhs = tp1.tile([P, LC], bf16, tag="rhs", name="rhs")
                            nc.sync.dma_start(rhs[:],
                                              hT.ap()[kt * P:(kt + 1) * P, cols])
                            for s in range(2):
                                wcol = (j * 2 + s) * P
                                nc.tensor.matmul(
                                    acc_ps[s][:],
                                    w_in_sb[:, kt, wcol:wcol + P],
                                    rhs[:],
                                    start=(kt == 0), stop=(kt == NKT - 1))
                        # x -> SBUF bf16 (ScalarE evac with cast)
                        nc.scalar.copy(x16[j][:, cols], acc_ps[0][:])
                        # z -> silu(z) in bf16 -> DRAM spill
                        sg = tp1.tile([P, LC], f32, tag="zsg", name="zsg")
                        nc.scalar.activation(sg[:], acc_ps[1][:], AF.Sigmoid)
                        z16 = tp1.tile([P, LC], bf16, tag="z16", name="z16")
                        nc.vector.tensor_tensor(z16[:], acc_ps[1][:], sg[:],
                                                op=ALU.mult)
                        nc.sync.dma_start(z16_dram.ap()[j, :, cols], z16[:])
                    conv_u(0, j)   # DVE conv overlaps next j's in_proj matmuls

                # dir-a x_proj partials + AllReduce, then dir-b conv during CC
                dbl_cc(0, dblps)
                for j in range(NPAIR):
                    conv_u(1, j)
                dbl_cc(1, dblps)

        # B/C rows -> bf16 DRAM staging (for replicated broadcast reads)
        with ExitStack() as pr:
            rp = pr.enter_context(tc.tile_pool(name="rowstage", bufs=1))
            for dr in range(2):
                rt = rp.tile([2 * DS, L], bf16, tag=f"rows16_{dr}",
                             name=f"rows16_{dr}")
                nc.gpsimd.dma_start(rt[:], cc_out.ap()[dr, DTR:E, :])
                nc.sync.dma_start(rows_dram.ap()[dr], rt[:])

        # =========== Phase 2: delta, scans, m accumulation, out_proj =========
        with ExitStack() as p2:
            dp2 = p2.enter_context(tc.tile_pool(name="p2d", bufs=1))
            bcp = p2.enter_context(tc.tile_pool(name="p2bc", bufs=2))
            ccp = p2.enter_context(tc.tile_pool(name="p2cc", bufs=1))
            wkp = p2.enter_context(tc.tile_pool(name="p2wk", bufs=2))

            for dr in range(2):
                with ExitStack() as pd:
                    dtp = pd.enter_context(tc.tile_pool(name="p2dt", bufs=1))
                    psd = pd.enter_context(
                        tc.tile_pool(name="dtps", bufs=2, space="PSUM"))
                    dtlow = dtp.tile([DTR, L], bf16, tag="dtlow", name="dtlow")
                    nc.gpsimd.dma_start(dtlow[:], cc_out.ap()[dr, 0:DTR, :])
                    delta = {}
                    for j in range(NPAIR):
                        dlt = dp2.tile([P, L], bf16, tag=f"delta_{j}",
                                       name=f"delta_{j}")
                        # softplus = ln(1 + exp(x)); batch Exps then one Ln to
                        # avoid per-chunk activation-table reloads
                        e32 = dtp.tile([P, L], f32, tag="e32", name="e32")
                        for lc in range(NLC):
                            c0, c1 = lc * LC, (lc + 1) * LC
                            dps = psd.tile([P, LC], f32, tag="dtps", name="dtps")
                            nc.tensor.matmul(
                                dps[:], w_dt_sb[:, dr, j, :], dtlow[:, c0:c1],
                                start=True, stop=True)
                            nc.scalar.activation(e32[:, c0:c1], dps[:], AF.Exp,
                                                 bias=dtb_sb[:, dr, j:j + 1])
                        nc.scalar.activation(dlt[:], e32[:], AF.Ln, bias=1.0)
                        delta[j] = dlt

                # u for this dir (reloaded transiently from DRAM spill)
                du = {}
                for j in range(NPAIR):
                    ut = dp2.tile([P, L], bf16, tag="uload", name="uload")
                    nc.sync.dma_start(ut[:], u16_dram.ap()[dr, j])
                    duj = dp2.tile([P, L], bf16, tag=f"du_{j}", name=f"du_{j}")
                    nc.vector.tensor_tensor(duj[:], delta[j][:], ut[:],
                                            op=ALU.mult)
                    du[j] = duj

                for n in range(DS):
                    Brep = bcp.tile([P, L], bf16, tag="Brep", name="Brep")
                    nc.sync.dma_start(
                        Brep[:], rows_dram.ap()[dr, n:n + 1, :].broadcast_to((P, L)))
                    Crep = ccp.tile([P, L], bf16, tag="Crep", name="Crep")
                    nc.sync.dma_start(
                        Crep[:], rows_dram.ap()[dr, DS + n:DS + n + 1, :]
                        .broadcast_to((P, L)))
                    for j in range(NPAIR):
                        dA = wkp.tile([P, L], bf16, tag="dA", name="dA")
                        nc.scalar.activation(dA[:], delta[j][:], AF.Exp,
                                             scale=Acol_sb[:, dr, j, n:n + 1])
                        d1 = wkp.tile([P, L], bf16, tag="d1", name="d1")
                        nc.vector.tensor_tensor(d1[:], du[j][:], Brep[:],
                                                op=ALU.mult)
                        h = wkp.tile([P, L], bf16, tag="h", name="h")
                        if dr == 0:
                            nc.vector.tensor_tensor_scan(
                                h[:], dA[:], d1[:], 0.0,
                                op0=ALU.mult, op1=ALU.add)
                        else:
                            nc.vector.tensor_tensor_scan(
                                h[:, ::-1], dA[:, ::-1], d1[:, ::-1], 0.0,
                                op0=ALU.mult, op1=ALU.add)
                        nc.vector.tensor_tensor(h[:], h[:], Crep[:], op=ALU.mult)
                        nc.vector.tensor_tensor(macc[dr, j][:], macc[dr, j][:],
                                                h[:], op=ALU.add)

        # --- gate by silu(z), out_proj (bf16), store ---
        if True:
            with ExitStack() as po:
                gp = po.enter_context(tc.tile_pool(name="p2g", bufs=1))
                pso = po.enter_context(
                    tc.tile_pool(name="outps", bufs=2, space="PSUM"))
                tpo = po.enter_context(tc.tile_pool(name="p2o", bufs=2))
                ygs = []
                for j in range(NPAIR):
                    zt = gp.tile([P, L], bf16, tag=f"zq_{j}", name=f"zq_{j}")
                    nc.sync.dma_start(zt[:], z16_dram.ap()[j])
                    ms = gp.tile([P, L], bf16, tag=f"ms_{j}", name=f"ms_{j}")
                    nc.vector.tensor_tensor(ms[:], macc[0, j][:], macc[1, j][:],
                                            op=ALU.add)
                    yg = gp.tile([P, L], bf16, tag=f"yg_{j}", name=f"yg_{j}")
                    nc.vector.tensor_tensor(yg[:], ms[:], zt[:], op=ALU.mult)
                    ygs.append(yg)
                for ot in range(D // P):
                    for lc in range(NLC):
                        c0 = lc * LC
                        ops_ = pso.tile([P, LC], f32, tag="outps", name="outps")
                        for j in range(NPAIR):
                            nc.tensor.matmul(
                                ops_[:],
                                w_out_sb[:, j, ot * P:(ot + 1) * P],
                                ygs[j][:, c0:c0 + LC],
                                start=(j == 0), stop=(j == NPAIR - 1))
                        osb = tpo.tile([P, LC], f32, tag="osb", name="osb")
                        nc.scalar.copy(osb[:], ops_[:])
                        nc.sync.dma_start(
                            out_part.ap()[ot * P:(ot + 1) * P, c0:c0 + LC],
                            osb[:])
    nc.compile()
    return nc


def _prep_core_inputs(inputs, core):
    """Host-side slicing/transposition of full inputs for one core."""
    b = core // 4
    dtiles = [(core % 4) * NPAIR + k for k in range(NPAIR)]
    chans = np.concatenate([np.arange(dt * P, (dt + 1) * P) for dt in dtiles])

    hid = np.asarray(inputs['hidden_states'])
    w_in_full = np.asarray(inputs['in_proj_w'])
    w_out_full = np.asarray(inputs['out_proj_w'])

    per_dir = {}
    for d, sfx in enumerate(('a', 'b')):
        per_dir[d] = dict(
            cw=np.asarray(inputs[f'conv_w_{sfx}'])[chans],
            cb=np.asarray(inputs[f'conv_b_{sfx}'])[chans],
            xp=np.asarray(inputs[f'x_proj_{sfx}_w'])[:, chans],
            dtp=np.asarray(inputs[f'dt_proj_{sfx}_w'])[chans],
            dtb=np.asarray(inputs[f'dt_bias_{sfx}'])[chans],
            A=-np.exp(np.asarray(inputs[f'A_{sfx}_log'])[chans]),
            Dv=np.asarray(inputs[f'D_{sfx}'])[chans],
        )

    w_in_cols = np.empty((D, 2 * NPAIR * P), np.float32)
    for j in range(NPAIR):
        ch_j = chans[j * P:(j + 1) * P]
        w_in_cols[:, (2 * j) * P:(2 * j + 1) * P] = w_in_full[ch_j].T
        w_in_cols[:, (2 * j + 1) * P:(2 * j + 2) * P] = w_in_full[DI + ch_j].T

    convw = np.empty((2, NPAIR, P, KC), np.float32)
    for d in range(2):
        cw = per_dir[d]['cw'].reshape(NPAIR, P, KC)
        if d == 0:
            convw[d] = cw
        else:
            convw[d] = cw[:, :, ::-1]       # reversed taps for backward conv

    out = {
        'hT': np.ascontiguousarray(hid[b].T).astype(bfnp),
        'w_in': np.ascontiguousarray(w_in_cols).astype(bfnp),
        'convw': np.ascontiguousarray(convw),
        'convb': np.ascontiguousarray(
            np.stack([per_dir[d]['cb'].reshape(NPAIR, P) for d in range(2)])
        ).astype(np.float32),
        'w_xp': np.ascontiguousarray(
            np.stack([per_dir[d]['xp'].T for d in range(2)])).astype(bfnp),
        'w_dt': np.ascontiguousarray(
            np.stack([per_dir[d]['dtp'].reshape(NPAIR, P, DTR)
                      .transpose(0, 2, 1) for d in range(2)])).astype(bfnp),
        'dt_bias': np.ascontiguousarray(
            np.stack([per_dir[d]['dtb'].reshape(NPAIR, P) for d in range(2)])
        ).astype(np.float32),
        'Acol': np.ascontiguousarray(
            np.stack([per_dir[d]['A'].reshape(NPAIR, P, DS) for d in range(2)])
        ).astype(np.float32),
        'Dvec': np.ascontiguousarray(
            np.stack([per_dir[d]['Dv'].reshape(NPAIR, P) for d in range(2)])
        ).astype(np.float32),
        'w_out': np.ascontiguousarray(
            w_out_full[:, chans].T.reshape(NPAIR, P, D)).astype(bfnp),
    }
    return out


_module_cache = {}


def _get_module():
    if 'nc' not in _module_cache:
        _module_cache['nc'] = build_module()
    return _module_cache['nc']


def kernel(**inputs):
    nc = _get_module()
    in_maps = [_prep_core_inputs(inputs, c) for c in range(NCORES)]
    res = run_bass_kernel_spmd(nc, in_maps, list(range(NCORES)))
    out = np.zeros((B, L, D), np.float32)
    for c in range(NCORES):
        out[c // 4] += res.results[c]['out_part'].T
    return out


# revision 28
# speedup vs baseline: 1.0450x; 1.0450x over previous
"""Bidirectional Mamba TRN2 kernel (8 NeuronCores, SPMD) — v2.

Sharding: core c owns batch b = c//4 and dtiles (c%4)*3 + {0,1,2} (3 tiles of
128 d_inner channels), BOTH scan directions. x_proj partials are AllReduced
per-batch (groups {0..3},{4..7}); out_proj partials summed on the host.

v2 structure (vs v1): full-L phase 2 (one 4096-long scan per (n,j,dir); no
quartering, no m_spill, no scan-state chaining), u/z kept on-chip or in small
bf16 spills, B/C rows broadcast by DMA from a bf16 DRAM staging row
(replicated-partition read) instead of GPSIMD partition_broadcast, the
16-state output accumulation m runs on GPSIMD in fp32 (DVE does only
d1/scan/hc), D-terms via ScalarE scale-copy, and every matmul is bf16.
"""
import numpy as np
from contextlib import ExitStack

import ml_dtypes
import concourse.bass as bass
import concourse.bacc as bacc
import concourse.tile as tile
from concourse import mybir, library_config
from concourse.bass_utils import run_bass_kernel_spmd

B, L, D = 2, 4096, 768
DI, DS, DTR, KC = 1536, 16, 48, 4
NCORES = 8
NPAIR = 3                 # dtiles per core
P = 128
NKT = D // P              # 6 K-tiles for in_proj
LC = 512                  # matmul free-dim chunk
NLC = L // LC             # 8
E = DTR + 2 * DS          # 80

f32 = mybir.dt.float32
bf16 = mybir.dt.bfloat16
ALU = mybir.AluOpType
AF = mybir.ActivationFunctionType
bfnp = ml_dtypes.bfloat16


def build_module():
    nc = bacc.Bacc("TRN2", target_bir_lowering=False, debug=False,
                   num_devices=NCORES)

    # ---- external inputs (per-core data; same tensor names on all cores) ----
    hT = nc.dram_tensor("hT", [D, L], bf16, kind="ExternalInput")
    w_in = nc.dram_tensor("w_in", [D, 2 * NPAIR * P], bf16, kind="ExternalInput")
    convw = nc.dram_tensor("convw", [2, NPAIR, P, KC], f32, kind="ExternalInput")
    convb = nc.dram_tensor("convb", [2, NPAIR, P], f32, kind="ExternalInput")
    w_xp = nc.dram_tensor("w_xp", [2, NPAIR * P, E], bf16, kind="ExternalInput")
    w_dt = nc.dram_tensor("w_dt", [2, NPAIR, DTR, P], bf16, kind="ExternalInput")
    dt_bias = nc.dram_tensor("dt_bias", [2, NPAIR, P], f32, kind="ExternalInput")
    Acol = nc.dram_tensor("Acol", [2, NPAIR, P, DS], f32, kind="ExternalInput")
    Dvec = nc.dram_tensor("Dvec", [2, NPAIR, P], f32, kind="ExternalInput")
    w_out = nc.dram_tensor("w_out", [NPAIR, P, D], bf16, kind="ExternalInput")
    out_part = nc.dram_tensor("out_part", [D, L], f32, kind="ExternalOutput")

    # ---- internal DRAM ----
    z16_dram = nc.dram_tensor("z16_dram", [NPAIR, P, L], bf16)
    u16_dram = nc.dram_tensor("u16_dram", [2, NPAIR, P, L], bf16)
    cc_in = nc.dram_tensor("cc_in", [2, E, L], f32)
    cc_out = nc.dram_tensor("cc_out", [2, E, L], f32)
    rows_dram = nc.dram_tensor("rows_dram", [2, 2 * DS, L], bf16)

    with tile.TileContext(nc) as tc, ExitStack() as top:
        wp = top.enter_context(tc.tile_pool(name="weights", bufs=1))
        pm = top.enter_context(tc.tile_pool(name="persist", bufs=1))

        nc.gpsimd.load_library(library_config.proxy)

        # ---- long-lived weights ----
        w_dt_sb = wp.tile([DTR, 2, NPAIR, P], bf16, tag="w_dt", name="w_dt")
        nc.sync.dma_start(w_dt_sb[:], w_dt.ap().rearrange("d j r p -> r d j p"))
        dtb_sb = wp.tile([P, 2, NPAIR], f32, tag="dtb", name="dtb")
        nc.sync.dma_start(dtb_sb[:], dt_bias.ap().rearrange("d j p -> p d j"))
        Acol_sb = wp.tile([P, 2, NPAIR, DS], f32, tag="Acol", name="Acol")
        nc.sync.dma_start(Acol_sb[:], Acol.ap().rearrange("d j p n -> p d j n"))
        D_sb = wp.tile([P, 2, NPAIR], f32, tag="Dsb", name="Dsb")
        nc.sync.dma_start(D_sb[:], Dvec.ap().rearrange("d j p -> p d j"))
        w_out_sb = wp.tile([P, NPAIR, D], bf16, tag="w_out", name="w_out")
        nc.sync.dma_start(w_out_sb[:], w_out.ap().rearrange("j p c -> p j c"))

        # persistent output accumulators, one per direction (bf16, DVE 2x)
        macc = {(dr, j): pm.tile([P, L], bf16, tag=f"m_{dr}_{j}",
                                 name=f"m_{dr}_{j}")
                for dr in range(2) for j in range(NPAIR)}

        # =========== Phase 1: in_proj, z-silu, conv, u, dbl partials ==========
        with ExitStack() as p1:
            wp1 = p1.enter_context(tc.tile_pool(name="p1w", bufs=1))
            x16p = p1.enter_context(tc.tile_pool(name="x16", bufs=1))
            tp1 = p1.enter_context(tc.tile_pool(name="p1tmp", bufs=2))
            bp1 = p1.enter_context(tc.tile_pool(name="p1big", bufs=1))
            dblp = p1.enter_context(tc.tile_pool(name="dblsb", bufs=1))

            w_in_sb = wp1.tile([P, NKT, 2 * NPAIR * P], bf16, tag="w_in", name="w_in")
            nc.sync.dma_start(w_in_sb[:],
                              w_in.ap().rearrange("(kt p) c -> p kt c", p=P))
            convw_sb = wp1.tile([P, 2, NPAIR, KC], f32, tag="convw", name="convw")
            nc.sync.dma_start(convw_sb[:],
                              convw.ap().rearrange("d j p k -> p d j k"))
            convb_sb = wp1.tile([P, 2, NPAIR], f32, tag="convb", name="convb")
            nc.sync.dma_start(convb_sb[:],
                              convb.ap().rearrange("d j p -> p d j"))
            w_xp_sb = wp1.tile([P, 2, NPAIR, E], bf16, tag="w_xp", name="w_xp")
            nc.sync.dma_start(w_xp_sb[:],
                              w_xp.ap().rearrange("d (j p) e -> p d j e", p=P))

            x16 = [x16p.tile([P, L], bf16, tag=f"x16_{j}", name=f"x16_{j}")
                   for j in range(NPAIR)]
            u16 = {(dr, j): x16p.tile([P, L], bf16, tag=f"u16_{dr}_{j}",
                                      name=f"u16_{dr}_{j}")
                   for dr in range(2) for j in range(NPAIR)}

            def conv_u(dr, j):
                """Depthwise causal conv + silu -> u16[dr, j]; init m D-term."""
                acc = bp1.tile([P, L], bf16, tag="cacc", name="cacc")
                w = lambda k: convw_sb[:, dr, j, k:k + 1]
                if dr == 0:   # taps k read x[t-3+k]
                    nc.vector.tensor_scalar(acc[:], x16[j][:], w(3),
                                            convb_sb[:, dr, j:j + 1],
                                            op0=ALU.mult, op1=ALU.add)
                    for k in range(3):
                        sh = 3 - k
                        nc.vector.scalar_tensor_tensor(
                            acc[:, sh:L], x16[j][:, 0:L - sh], w(k),
                            acc[:, sh:L], op0=ALU.mult, op1=ALU.add)
                else:         # host-reversed taps jj read x[t+jj]
                    nc.vector.tensor_scalar(acc[:], x16[j][:], w(0),
                                            convb_sb[:, dr, j:j + 1],
                                            op0=ALU.mult, op1=ALU.add)
                    for jj in range(1, 4):
                        nc.vector.scalar_tensor_tensor(
                            acc[:, 0:L - jj], x16[j][:, jj:L], w(jj),
                            acc[:, 0:L - jj], op0=ALU.mult, op1=ALU.add)
                sg = bp1.tile([P, L], f32, tag="usg", name="usg")
                nc.scalar.activation(sg[:], acc[:], AF.Sigmoid)
                u = u16[dr, j]
                nc.vector.tensor_tensor(u[:], acc[:], sg[:], op=ALU.mult)
                nc.sync.dma_start(u16_dram.ap()[dr, j], u[:])
                nc.scalar.activation(macc[dr, j][:], u[:], AF.Copy,
                                     scale=D_sb[:, dr, j:j + 1])

            def dbl_cc(dr, dblps):
                """x_proj partial matmuls for one dir, then its AllReduce."""
                for lc in range(NLC):
                    cols = slice(lc * LC, (lc + 1) * LC)
                    dps = dblps.tile([E, LC], f32, tag="dblps", name="dblps")
                    for j in range(NPAIR):
                        nc.tensor.matmul(
                            dps[:], w_xp_sb[:, dr, j, :],
                            u16[dr, j][:, cols],
                            start=(j == 0), stop=(j == NPAIR - 1))
                    dsb = tp1.tile([E, LC], f32, tag="dsb", name="dsb")
                    nc.scalar.copy(dsb[:], dps[:])
                    nc.sync.dma_start(cc_in.ap()[dr, :, cols], dsb[:])
                nc.gpsimd.collective_compute(
                    "AllReduce", ALU.add,
                    replica_groups=[[0, 1, 2, 3], [4, 5, 6, 7]],
                    ins=[cc_in.ap()[dr]], outs=[cc_out.ap()[dr]])

            # --- in_proj j-major (bf16 matmuls), conv_a interleaved per j ---
            with ExitStack() as s1:
                xzps = s1.enter_context(
                    tc.tile_pool(name="xzps", bufs=1, space="PSUM"))
                dblps = s1.enter_context(
                    tc.tile_pool(name="dblps", bufs=2, space="PSUM"))
                # 2 column-chunks in flight (4 PSUM banks) so consecutive
                # matmuls never target the same accumulating bank
                acc_ps = [[xzps.tile([P, LC], f32, tag=f"xz{c}{s}",
                                     name=f"xz{c}{s}") for s in range(2)]
                          for c in range(2)]
                for j in range(NPAIR):
                    for lcp in range(NLC // 2):
                        base = lcp * 2
                        for kt in range(NKT):
                            rhs2 = []
                            for c in range(2):
                                lc = base + c
                                cols = slice(lc * LC, (lc + 1) * LC)
                                rhs = tp1.tile([P, LC], bf16, tag=f"rhs{c}",
                                               name=f"rhs{c}")
                                nc.sync.dma_start(
                                    rhs[:], hT.ap()[kt * P:(kt + 1) * P, cols])
                                rhs2.append(rhs)
                            for s in range(2):
                                wcol = (j * 2 + s) * P
                                for c in range(2):
                                    nc.tensor.matmul(
                                        acc_ps[c][s][:],
                                        w_in_sb[:, kt, wcol:wcol + P],
                                        rhs2[c][:],
                                        start=(kt == 0), stop=(kt == NKT - 1))
                        for c in range(2):
                            lc = base + c
                            cols = slice(lc * LC, (lc + 1) * LC)
                            # x -> SBUF bf16 (ScalarE evac with cast)
                            nc.scalar.copy(x16[j][:, cols], acc_ps[c][0][:])
                            # z -> silu(z) in bf16 -> DRAM spill
                            sg = tp1.tile([P, LC], f32, tag="zsg", name="zsg")
                            nc.scalar.activation(sg[:], acc_ps[c][1][:], AF.Sigmoid)
                            z16 = tp1.tile([P, LC], bf16, tag="z16", name="z16")
                            nc.vector.tensor_tensor(z16[:], acc_ps[c][1][:], sg[:],
                                                    op=ALU.mult)
                            nc.sync.dma_start(z16_dram.ap()[j, :, cols], z16[:])
                    conv_u(0, j)   # DVE conv overlaps next j's in_proj matmuls

                # dir-a x_proj partials + AllReduce, then dir-b conv during CC
                dbl_cc(0, dblps)
                for j in range(NPAIR):
                    conv_u(1, j)
                dbl_cc(1, dblps)

        # B/C rows -> bf16 DRAM staging (for replicated broadcast reads)
        with ExitStack() as pr:
            rp = pr.enter_context(tc.tile_pool(name="rowstage", bufs=1))
            for dr in range(2):
                rt = rp.tile([2 * DS, L], bf16, tag=f"rows16_{dr}",
                             name=f"rows16_{dr}")
                nc.gpsimd.dma_start(rt[:], cc_out.ap()[dr, DTR:E, :])
                nc.sync.dma_start(rows_dram.ap()[dr], rt[:])

        # =========== Phase 2: delta, scans, m accumulation, out_proj =========
        with ExitStack() as p2:
            dp2 = p2.enter_context(tc.tile_pool(name="p2d", bufs=1))
            bcp = p2.enter_context(tc.tile_pool(name="p2bc", bufs=2))
            ccp = p2.enter_context(tc.tile_pool(name="p2cc", bufs=1))
            wkp = p2.enter_context(tc.tile_pool(name="p2wk", bufs=2))

            for dr in range(2):
                with ExitStack() as pd:
                    dtp = pd.enter_context(tc.tile_pool(name="p2dt", bufs=1))
                    psd = pd.enter_context(
                        tc.tile_pool(name="dtps", bufs=2, space="PSUM"))
                    dtlow = dtp.tile([DTR, L], bf16, tag="dtlow", name="dtlow")
                    nc.gpsimd.dma_start(dtlow[:], cc_out.ap()[dr, 0:DTR, :])
                    delta = {}
                    for j in range(NPAIR):
                        dlt = dp2.tile([P, L], bf16, tag=f"delta_{j}",
                                       name=f"delta_{j}")
                        # softplus = ln(1 + exp(x)); batch Exps then one Ln to
                        # avoid per-chunk activation-table reloads
                        e32 = dtp.tile([P, L], f32, tag="e32", name="e32")
                        for lc in range(NLC):
                            c0, c1 = lc * LC, (lc + 1) * LC
                            dps = psd.tile([P, LC], f32, tag="dtps", name="dtps")
                            nc.tensor.matmul(
                                dps[:], w_dt_sb[:, dr, j, :], dtlow[:, c0:c1],
                                start=True, stop=True)
                            nc.scalar.activation(e32[:, c0:c1], dps[:], AF.Exp,
                                                 bias=dtb_sb[:, dr, j:j + 1])
                        nc.scalar.activation(dlt[:], e32[:], AF.Ln, bias=1.0)
                        delta[j] = dlt

                # u for this dir (reloaded transiently from DRAM spill)
                du = {}
                for j in range(NPAIR):
                    ut = dp2.tile([P, L], bf16, tag="uload", name="uload")
                    nc.sync.dma_start(ut[:], u16_dram.ap()[dr, j])
                    duj = dp2.tile([P, L], bf16, tag=f"du_{j}", name=f"du_{j}")
                    nc.vector.tensor_tensor(duj[:], delta[j][:], ut[:],
                                            op=ALU.mult)
                    du[j] = duj

                for n in range(DS):
                    Brep = bcp.tile([P, L], bf16, tag="Brep", name="Brep")
                    nc.sync.dma_start(
                        Brep[:], rows_dram.ap()[dr, n:n + 1, :].broadcast_to((P, L)))
                    Crep = ccp.tile([P, L], bf16, tag="Crep", name="Crep")
                    nc.sync.dma_start(
                        Crep[:], rows_dram.ap()[dr, DS + n:DS + n + 1, :]
                        .broadcast_to((P, L)))
                    for j in range(NPAIR):
                        dA = wkp.tile([P, L], bf16, tag="dA", name="dA")
                        nc.scalar.activation(dA[:], delta[j][:], AF.Exp,
                                             scale=Acol_sb[:, dr, j, n:n + 1])
                        d1 = wkp.tile([P, L], bf16, tag="d1", name="d1")
                        nc.vector.tensor_tensor(d1[:], du[j][:], Brep[:],
                                                op=ALU.mult)
                        h = wkp.tile([P, L], bf16, tag="h", name="h")
                        if dr == 0:
                            nc.vector.tensor_tensor_scan(
                                h[:], dA[:], d1[:], 0.0,
                                op0=ALU.mult, op1=ALU.add)
                        else:
                            nc.vector.tensor_tensor_scan(
                                h[:, ::-1], dA[:, ::-1], d1[:, ::-1], 0.0,
                                op0=ALU.mult, op1=ALU.add)
                        nc.vector.tensor_tensor(h[:], h[:], Crep[:], op=ALU.mult)
                        nc.vector.tensor_tensor(macc[dr, j][:], macc[dr, j][:],
                                                h[:], op=ALU.add)

        # --- gate by silu(z), out_proj (bf16), store ---
        if True:
            with ExitStack() as po:
                gp = po.enter_context(tc.tile_pool(name="p2g", bufs=1))
                pso = po.enter_context(
                    tc.tile_pool(name="outps", bufs=2, space="PSUM"))
                tpo = po.enter_context(tc.tile_pool(name="p2o", bufs=2))
                ygs = []
                for j in range(NPAIR):
                    zt = gp.tile([P, L], bf16, tag=f"zq_{j}", name=f"zq_{j}")
                    nc.sync.dma_start(zt[:], z16_dram.ap()[j])
                    ms = gp.tile([P, L], bf16, tag=f"ms_{j}", name=f"ms_{j}")
                    nc.vector.tensor_tensor(ms[:], macc[0, j][:], macc[1, j][:],
                                            op=ALU.add)
                    yg = gp.tile([P, L], bf16, tag=f"yg_{j}", name=f"yg_{j}")
                    nc.vector.tensor_tensor(yg[:], ms[:], zt[:], op=ALU.mult)
                    ygs.append(yg)
                for ot in range(D // P):
                    for lc in range(NLC):
                        c0 = lc * LC
                        ops_ = pso.tile([P, LC], f32, tag="outps", name="outps")
                        for j in range(NPAIR):
                            nc.tensor.matmul(
                                ops_[:],
                                w_out_sb[:, j, ot * P:(ot + 1) * P],
                                ygs[j][:, c0:c0 + LC],
                                start=(j == 0), stop=(j == NPAIR - 1))
                        osb = tpo.tile([P, LC], f32, tag="osb", name="osb")
                        nc.scalar.copy(osb[:], ops_[:])
                        nc.sync.dma_start(
                            out_part.ap()[ot * P:(ot + 1) * P, c0:c0 + LC],
                            osb[:])
    nc.compile()
    return nc


def _prep_core_inputs(inputs, core):
    """Host-side slicing/transposition of full inputs for one core."""
    b = core // 4
    dtiles = [(core % 4) * NPAIR + k for k in range(NPAIR)]
    chans = np.concatenate([np.arange(dt * P, (dt + 1) * P) for dt in dtiles])

    hid = np.asarray(inputs['hidden_states'])
    w_in_full = np.asarray(inputs['in_proj_w'])
    w_out_full = np.asarray(inputs['out_proj_w'])

    per_dir = {}
    for d, sfx in enumerate(('a', 'b')):
        per_dir[d] = dict(
            cw=np.asarray(inputs[f'conv_w_{sfx}'])[chans],
            cb=np.asarray(inputs[f'conv_b_{sfx}'])[chans],
            xp=np.asarray(inputs[f'x_proj_{sfx}_w'])[:, chans],
            dtp=np.asarray(inputs[f'dt_proj_{sfx}_w'])[chans],
            dtb=np.asarray(inputs[f'dt_bias_{sfx}'])[chans],
            A=-np.exp(np.asarray(inputs[f'A_{sfx}_log'])[chans]),
            Dv=np.asarray(inputs[f'D_{sfx}'])[chans],
        )

    w_in_cols = np.empty((D, 2 * NPAIR * P), np.float32)
    for j in range(NPAIR):
        ch_j = chans[j * P:(j + 1) * P]
        w_in_cols[:, (2 * j) * P:(2 * j + 1) * P] = w_in_full[ch_j].T
        w_in_cols[:, (2 * j + 1) * P:(2 * j + 2) * P] = w_in_full[DI + ch_j].T

    convw = np.empty((2, NPAIR, P, KC), np.float32)
    for d in range(2):
        cw = per_dir[d]['cw'].reshape(NPAIR, P, KC)
        if d == 0:
            convw[d] = cw
        else:
            convw[d] = cw[:, :, ::-1]       # reversed taps for backward conv

    out = {
        'hT': np.ascontiguousarray(hid[b].T).astype(bfnp),
        'w_in': np.ascontiguousarray(w_in_cols).astype(bfnp),
        'convw': np.ascontiguousarray(convw),
        'convb': np.ascontiguousarray(
            np.stack([per_dir[d]['cb'].reshape(NPAIR, P) for d in range(2)])
        ).astype(np.float32),
        'w_xp': np.ascontiguousarray(
            np.stack([per_dir[d]['xp'].T for d in range(2)])).astype(bfnp),
        'w_dt': np.ascontiguousarray(
            np.stack([per_dir[d]['dtp'].reshape(NPAIR, P, DTR)
                      .transpose(0, 2, 1) for d in range(2)])).astype(bfnp),
        'dt_bias': np.ascontiguousarray(
            np.stack([per_dir[d]['dtb'].reshape(NPAIR, P) for d in range(2)])
        ).astype(np.float32),
        'Acol': np.ascontiguousarray(
            np.stack([per_dir[d]['A'].reshape(NPAIR, P, DS) for d in range(2)])
        ).astype(np.float32),
        'Dvec': np.ascontiguousarray(
            np.stack([per_dir[d]['Dv'].reshape(NPAIR, P) for d in range(2)])
        ).astype(np.float32),
        'w_out': np.ascontiguousarray(
            w_out_full[:, chans].T.reshape(NPAIR, P, D)).astype(bfnp),
    }
    return out


_module_cache = {}


def _get_module():
    if 'nc' not in _module_cache:
        _module_cache['nc'] = build_module()
    return _module_cache['nc']


def kernel(**inputs):
    nc = _get_module()
    in_maps = [_prep_core_inputs(inputs, c) for c in range(NCORES)]
    res = run_bass_kernel_spmd(nc, in_maps, list(range(NCORES)))
    out = np.zeros((B, L, D), np.float32)
    for c in range(NCORES):
        out[c // 4] += res.results[c]['out_part'].T
    return out


# revision 29
# speedup vs baseline: 1.1002x; 1.0529x over previous
"""Bidirectional Mamba TRN2 kernel (8 NeuronCores, SPMD) — v2.

Sharding: core c owns batch b = c//4 and dtiles (c%4)*3 + {0,1,2} (3 tiles of
128 d_inner channels), BOTH scan directions. x_proj partials are AllReduced
per-batch (groups {0..3},{4..7}); out_proj partials summed on the host.

v2 structure (vs v1): full-L phase 2 (one 4096-long scan per (n,j,dir); no
quartering, no m_spill, no scan-state chaining), u/z kept on-chip or in small
bf16 spills, B/C rows broadcast by DMA from a bf16 DRAM staging row
(replicated-partition read) instead of GPSIMD partition_broadcast, the
16-state output accumulation m runs on GPSIMD in fp32 (DVE does only
d1/scan/hc), D-terms via ScalarE scale-copy, and every matmul is bf16.
"""
import numpy as np
from contextlib import ExitStack

import ml_dtypes
import concourse.bass as bass
import concourse.bacc as bacc
import concourse.tile as tile
from concourse import mybir, library_config
from concourse.bass_utils import run_bass_kernel_spmd

B, L, D = 2, 4096, 768
DI, DS, DTR, KC = 1536, 16, 48, 4
NCORES = 8
NPAIR = 3                 # dtiles per core
P = 128
NKT = D // P              # 6 K-tiles for in_proj
LC = 512                  # matmul free-dim chunk
NLC = L // LC             # 8
E = DTR + 2 * DS          # 80

f32 = mybir.dt.float32
bf16 = mybir.dt.bfloat16
ALU = mybir.AluOpType
AF = mybir.ActivationFunctionType
bfnp = ml_dtypes.bfloat16


def build_module():
    nc = bacc.Bacc("TRN2", target_bir_lowering=False, debug=False,
                   num_devices=NCORES)

    # ---- external inputs (per-core data; same tensor names on all cores) ----
    hT = nc.dram_tensor("hT", [D, L], bf16, kind="ExternalInput")
    w_in = nc.dram_tensor("w_in", [D, 2 * NPAIR * P], bf16, kind="ExternalInput")
    convw = nc.dram_tensor("convw", [2, NPAIR, P, KC], f32, kind="ExternalInput")
    convb = nc.dram_tensor("convb", [2, NPAIR, P], f32, kind="ExternalInput")
    w_xp = nc.dram_tensor("w_xp", [2, NPAIR * P, E], bf16, kind="ExternalInput")
    w_dt = nc.dram_tensor("w_dt", [2, NPAIR, DTR, P], bf16, kind="ExternalInput")
    dt_bias = nc.dram_tensor("dt_bias", [2, NPAIR, P], f32, kind="ExternalInput")
    Acol = nc.dram_tensor("Acol", [2, NPAIR, P, DS], f32, kind="ExternalInput")
    Dvec = nc.dram_tensor("Dvec", [2, NPAIR, P], f32, kind="ExternalInput")
    w_out = nc.dram_tensor("w_out", [NPAIR, P, D], bf16, kind="ExternalInput")
    out_part = nc.dram_tensor("out_part", [D, L], f32, kind="ExternalOutput")

    # ---- internal DRAM ----
    z16_dram = nc.dram_tensor("z16_dram", [NPAIR, P, L], bf16)
    u16_dram = nc.dram_tensor("u16_dram", [2, NPAIR, P, L], bf16)
    cc_in = nc.dram_tensor("cc_in", [2, E, L], bf16)
    cc_out = nc.dram_tensor("cc_out", [2, E, L], bf16)

    with tile.TileContext(nc) as tc, ExitStack() as top:
        wp = top.enter_context(tc.tile_pool(name="weights", bufs=1))
        pm = top.enter_context(tc.tile_pool(name="persist", bufs=1))

        nc.gpsimd.load_library(library_config.proxy)

        # ---- long-lived weights ----
        w_dt_sb = wp.tile([DTR, 2, NPAIR, P], bf16, tag="w_dt", name="w_dt")
        nc.sync.dma_start(w_dt_sb[:], w_dt.ap().rearrange("d j r p -> r d j p"))
        dtb_sb = wp.tile([P, 2, NPAIR], f32, tag="dtb", name="dtb")
        nc.sync.dma_start(dtb_sb[:], dt_bias.ap().rearrange("d j p -> p d j"))
        Acol_sb = wp.tile([P, 2, NPAIR, DS], f32, tag="Acol", name="Acol")
        nc.sync.dma_start(Acol_sb[:], Acol.ap().rearrange("d j p n -> p d j n"))
        D_sb = wp.tile([P, 2, NPAIR], f32, tag="Dsb", name="Dsb")
        nc.sync.dma_start(D_sb[:], Dvec.ap().rearrange("d j p -> p d j"))
        w_out_sb = wp.tile([P, NPAIR, D], bf16, tag="w_out", name="w_out")
        nc.sync.dma_start(w_out_sb[:], w_out.ap().rearrange("j p c -> p j c"))

        # persistent output accumulators, one per direction (bf16, DVE 2x)
        macc = {(dr, j): pm.tile([P, L], bf16, tag=f"m_{dr}_{j}",
                                 name=f"m_{dr}_{j}")
                for dr in range(2) for j in range(NPAIR)}

        # =========== Phase 1: in_proj, z-silu, conv, u, dbl partials ==========
        with ExitStack() as p1:
            wp1 = p1.enter_context(tc.tile_pool(name="p1w", bufs=1))
            x16p = p1.enter_context(tc.tile_pool(name="x16", bufs=1))
            tp1 = p1.enter_context(tc.tile_pool(name="p1tmp", bufs=2))
            bp1 = p1.enter_context(tc.tile_pool(name="p1big", bufs=1))
            dblp = p1.enter_context(tc.tile_pool(name="dblsb", bufs=1))

            w_in_sb = wp1.tile([P, NKT, 2 * NPAIR * P], bf16, tag="w_in", name="w_in")
            nc.sync.dma_start(w_in_sb[:],
                              w_in.ap().rearrange("(kt p) c -> p kt c", p=P))
            convw_sb = wp1.tile([P, 2, NPAIR, KC], f32, tag="convw", name="convw")
            nc.sync.dma_start(convw_sb[:],
                              convw.ap().rearrange("d j p k -> p d j k"))
            convb_sb = wp1.tile([P, 2, NPAIR], f32, tag="convb", name="convb")
            nc.sync.dma_start(convb_sb[:],
                              convb.ap().rearrange("d j p -> p d j"))
            w_xp_sb = wp1.tile([P, 2, NPAIR, E], bf16, tag="w_xp", name="w_xp")
            nc.sync.dma_start(w_xp_sb[:],
                              w_xp.ap().rearrange("d (j p) e -> p d j e", p=P))

            x16 = [x16p.tile([P, L], bf16, tag=f"x16_{j}", name=f"x16_{j}")
                   for j in range(NPAIR)]
            u16 = {(dr, j): x16p.tile([P, L], bf16, tag=f"u16_{dr}_{j}",
                                      name=f"u16_{dr}_{j}")
                   for dr in range(2) for j in range(NPAIR)}

            def conv_u(dr, j):
                """Depthwise causal conv + silu -> u16[dr, j]; init m D-term."""
                acc = bp1.tile([P, L], bf16, tag="cacc", name="cacc")
                w = lambda k: convw_sb[:, dr, j, k:k + 1]
                if dr == 0:   # taps k read x[t-3+k]
                    nc.vector.tensor_scalar(acc[:], x16[j][:], w(3),
                                            convb_sb[:, dr, j:j + 1],
                                            op0=ALU.mult, op1=ALU.add)
                    for k in range(3):
                        sh = 3 - k
                        nc.vector.scalar_tensor_tensor(
                            acc[:, sh:L], x16[j][:, 0:L - sh], w(k),
                            acc[:, sh:L], op0=ALU.mult, op1=ALU.add)
                else:         # host-reversed taps jj read x[t+jj]
                    nc.vector.tensor_scalar(acc[:], x16[j][:], w(0),
                                            convb_sb[:, dr, j:j + 1],
                                            op0=ALU.mult, op1=ALU.add)
                    for jj in range(1, 4):
                        nc.vector.scalar_tensor_tensor(
                            acc[:, 0:L - jj], x16[j][:, jj:L], w(jj),
                            acc[:, 0:L - jj], op0=ALU.mult, op1=ALU.add)
                sg = bp1.tile([P, L], f32, tag="usg", name="usg")
                nc.scalar.activation(sg[:], acc[:], AF.Sigmoid)
                u = u16[dr, j]
                nc.vector.tensor_tensor(u[:], acc[:], sg[:], op=ALU.mult)
                nc.sync.dma_start(u16_dram.ap()[dr, j], u[:])
                nc.scalar.activation(macc[dr, j][:], u[:], AF.Copy,
                                     scale=D_sb[:, dr, j:j + 1])

            def dbl_cc(dr, dblps):
                """x_proj partial matmuls for one dir, then its AllReduce."""
                for lc in range(NLC):
                    cols = slice(lc * LC, (lc + 1) * LC)
                    dps = dblps.tile([E, LC], f32, tag="dblps", name="dblps")
                    for j in range(NPAIR):
                        nc.tensor.matmul(
                            dps[:], w_xp_sb[:, dr, j, :],
                            u16[dr, j][:, cols],
                            start=(j == 0), stop=(j == NPAIR - 1))
                    dsb = tp1.tile([E, LC], bf16, tag="dsb", name="dsb")
                    nc.scalar.copy(dsb[:], dps[:])
                    nc.sync.dma_start(cc_in.ap()[dr, :, cols], dsb[:])
                nc.gpsimd.collective_compute(
                    "AllReduce", ALU.add,
                    replica_groups=[[0, 1, 2, 3], [4, 5, 6, 7]],
                    ins=[cc_in.ap()[dr]], outs=[cc_out.ap()[dr]])

            # --- in_proj j-major (bf16 matmuls), conv_a interleaved per j ---
            with ExitStack() as s1:
                xzps = s1.enter_context(
                    tc.tile_pool(name="xzps", bufs=1, space="PSUM"))
                dblps = s1.enter_context(
                    tc.tile_pool(name="dblps", bufs=2, space="PSUM"))
                # 2 column-chunks in flight (4 PSUM banks) so consecutive
                # matmuls never target the same accumulating bank
                acc_ps = [[xzps.tile([P, LC], f32, tag=f"xz{c}{s}",
                                     name=f"xz{c}{s}") for s in range(2)]
                          for c in range(2)]
                for j in range(NPAIR):
                    for lcp in range(NLC // 2):
                        base = lcp * 2
                        for kt in range(NKT):
                            rhs2 = []
                            for c in range(2):
                                lc = base + c
                                cols = slice(lc * LC, (lc + 1) * LC)
                                rhs = tp1.tile([P, LC], bf16, tag=f"rhs{c}",
                                               name=f"rhs{c}")
                                nc.sync.dma_start(
                                    rhs[:], hT.ap()[kt * P:(kt + 1) * P, cols])
                                rhs2.append(rhs)
                            for s in range(2):
                                wcol = (j * 2 + s) * P
                                for c in range(2):
                                    nc.tensor.matmul(
                                        acc_ps[c][s][:],
                                        w_in_sb[:, kt, wcol:wcol + P],
                                        rhs2[c][:],
                                        start=(kt == 0), stop=(kt == NKT - 1))
                        for c in range(2):
                            lc = base + c
                            cols = slice(lc * LC, (lc + 1) * LC)
                            # x -> SBUF bf16 (ScalarE evac with cast)
                            nc.scalar.copy(x16[j][:, cols], acc_ps[c][0][:])
                            # z -> silu(z) in bf16 -> DRAM spill
                            sg = tp1.tile([P, LC], f32, tag="zsg", name="zsg")
                            nc.scalar.activation(sg[:], acc_ps[c][1][:], AF.Sigmoid)
                            z16 = tp1.tile([P, LC], bf16, tag="z16", name="z16")
                            nc.vector.tensor_tensor(z16[:], acc_ps[c][1][:], sg[:],
                                                    op=ALU.mult)
                            nc.sync.dma_start(z16_dram.ap()[j, :, cols], z16[:])
                    conv_u(0, j)   # DVE conv overlaps next j's in_proj matmuls

                # dir-a x_proj partials + AllReduce, then dir-b conv during CC
                dbl_cc(0, dblps)
                for j in range(NPAIR):
                    conv_u(1, j)
                dbl_cc(1, dblps)

        # =========== Phase 2: delta, scans, m accumulation, out_proj =========
        with ExitStack() as p2:
            dp2 = p2.enter_context(tc.tile_pool(name="p2d", bufs=1))
            bcp = p2.enter_context(tc.tile_pool(name="p2bc", bufs=2))
            ccp = p2.enter_context(tc.tile_pool(name="p2cc", bufs=1))
            wkp = p2.enter_context(tc.tile_pool(name="p2wk", bufs=2))

            for dr in range(2):
                with ExitStack() as pd:
                    dtp = pd.enter_context(tc.tile_pool(name="p2dt", bufs=1))
                    psd = pd.enter_context(
                        tc.tile_pool(name="dtps", bufs=2, space="PSUM"))
                    dtlow = dtp.tile([DTR, L], bf16, tag="dtlow", name="dtlow")
                    nc.sync.dma_start(dtlow[:], cc_out.ap()[dr, 0:DTR, :])
                    delta = {}
                    for j in range(NPAIR):
                        dlt = dp2.tile([P, L], bf16, tag=f"delta_{j}",
                                       name=f"delta_{j}")
                        # softplus = ln(1 + exp(x)); batch Exps then one Ln to
                        # avoid per-chunk activation-table reloads
                        e32 = dtp.tile([P, L], f32, tag="e32", name="e32")
                        for lc in range(NLC):
                            c0, c1 = lc * LC, (lc + 1) * LC
                            dps = psd.tile([P, LC], f32, tag="dtps", name="dtps")
                            nc.tensor.matmul(
                                dps[:], w_dt_sb[:, dr, j, :], dtlow[:, c0:c1],
                                start=True, stop=True)
                            nc.scalar.activation(e32[:, c0:c1], dps[:], AF.Exp,
                                                 bias=dtb_sb[:, dr, j:j + 1])
                        nc.scalar.activation(dlt[:], e32[:], AF.Ln, bias=1.0)
                        delta[j] = dlt

                # u for this dir (reloaded transiently from DRAM spill)
                du = {}
                for j in range(NPAIR):
                    ut = dp2.tile([P, L], bf16, tag="uload", name="uload")
                    nc.sync.dma_start(ut[:], u16_dram.ap()[dr, j])
                    duj = dp2.tile([P, L], bf16, tag=f"du_{j}", name=f"du_{j}")
                    nc.vector.tensor_tensor(duj[:], delta[j][:], ut[:],
                                            op=ALU.mult)
                    du[j] = duj

                for n in range(DS):
                    Brep = bcp.tile([P, L], bf16, tag="Brep", name="Brep")
                    nc.sync.dma_start(
                        Brep[:],
                        cc_out.ap()[dr, DTR + n:DTR + n + 1, :]
                        .broadcast_to((P, L)))
                    Crep = ccp.tile([P, L], bf16, tag="Crep", name="Crep")
                    nc.sync.dma_start(
                        Crep[:],
                        cc_out.ap()[dr, DTR + DS + n:DTR + DS + n + 1, :]
                        .broadcast_to((P, L)))
                    for j in range(NPAIR):
                        dA = wkp.tile([P, L], bf16, tag="dA", name="dA")
                        nc.scalar.activation(dA[:], delta[j][:], AF.Exp,
                                             scale=Acol_sb[:, dr, j, n:n + 1])
                        d1 = wkp.tile([P, L], bf16, tag="d1", name="d1")
                        nc.vector.tensor_tensor(d1[:], du[j][:], Brep[:],
                                                op=ALU.mult)
                        h = wkp.tile([P, L], bf16, tag="h", name="h")
                        if dr == 0:
                            nc.vector.tensor_tensor_scan(
                                h[:], dA[:], d1[:], 0.0,
                                op0=ALU.mult, op1=ALU.add)
                        else:
                            nc.vector.tensor_tensor_scan(
                                h[:, ::-1], dA[:, ::-1], d1[:, ::-1], 0.0,
                                op0=ALU.mult, op1=ALU.add)
                        nc.vector.tensor_tensor(h[:], h[:], Crep[:], op=ALU.mult)
                        nc.vector.tensor_tensor(macc[dr, j][:], macc[dr, j][:],
                                                h[:], op=ALU.add)

        # --- gate by silu(z), out_proj (bf16), store ---
        if True:
            with ExitStack() as po:
                gp = po.enter_context(tc.tile_pool(name="p2g", bufs=1))
                pso = po.enter_context(
                    tc.tile_pool(name="outps", bufs=2, space="PSUM"))
                tpo = po.enter_context(tc.tile_pool(name="p2o", bufs=2))
                ygs = []
                for j in range(NPAIR):
                    zt = gp.tile([P, L], bf16, tag=f"zq_{j}", name=f"zq_{j}")
                    nc.sync.dma_start(zt[:], z16_dram.ap()[j])
                    ms = gp.tile([P, L], bf16, tag=f"ms_{j}", name=f"ms_{j}")
                    nc.vector.tensor_tensor(ms[:], macc[0, j][:], macc[1, j][:],
                                            op=ALU.add)
                    yg = gp.tile([P, L], bf16, tag=f"yg_{j}", name=f"yg_{j}")
                    nc.vector.tensor_tensor(yg[:], ms[:], zt[:], op=ALU.mult)
                    ygs.append(yg)
                for ot in range(D // P):
                    for lc in range(NLC):
                        c0 = lc * LC
                        ops_ = pso.tile([P, LC], f32, tag="outps", name="outps")
                        for j in range(NPAIR):
                            nc.tensor.matmul(
                                ops_[:],
                                w_out_sb[:, j, ot * P:(ot + 1) * P],
                                ygs[j][:, c0:c0 + LC],
                                start=(j == 0), stop=(j == NPAIR - 1))
                        osb = tpo.tile([P, LC], f32, tag="osb", name="osb")
                        nc.scalar.copy(osb[:], ops_[:])
                        nc.sync.dma_start(
                            out_part.ap()[ot * P:(ot + 1) * P, c0:c0 + LC],
                            osb[:])
    nc.compile()
    return nc


def _prep_core_inputs(inputs, core):
    """Host-side slicing/transposition of full inputs for one core."""
    b = core // 4
    dtiles = [(core % 4) * NPAIR + k for k in range(NPAIR)]
    chans = np.concatenate([np.arange(dt * P, (dt + 1) * P) for dt in dtiles])

    hid = np.asarray(inputs['hidden_states'])
    w_in_full = np.asarray(inputs['in_proj_w'])
    w_out_full = np.asarray(inputs['out_proj_w'])

    per_dir = {}
    for d, sfx in enumerate(('a', 'b')):
        per_dir[d] = dict(
            cw=np.asarray(inputs[f'conv_w_{sfx}'])[chans],
            cb=np.asarray(inputs[f'conv_b_{sfx}'])[chans],
            xp=np.asarray(inputs[f'x_proj_{sfx}_w'])[:, chans],
            dtp=np.asarray(inputs[f'dt_proj_{sfx}_w'])[chans],
            dtb=np.asarray(inputs[f'dt_bias_{sfx}'])[chans],
            A=-np.exp(np.asarray(inputs[f'A_{sfx}_log'])[chans]),
            Dv=np.asarray(inputs[f'D_{sfx}'])[chans],
        )

    w_in_cols = np.empty((D, 2 * NPAIR * P), np.float32)
    for j in range(NPAIR):
        ch_j = chans[j * P:(j + 1) * P]
        w_in_cols[:, (2 * j) * P:(2 * j + 1) * P] = w_in_full[ch_j].T
        w_in_cols[:, (2 * j + 1) * P:(2 * j + 2) * P] = w_in_full[DI + ch_j].T

    convw = np.empty((2, NPAIR, P, KC), np.float32)
    for d in range(2):
        cw = per_dir[d]['cw'].reshape(NPAIR, P, KC)
        if d == 0:
            convw[d] = cw
        else:
            convw[d] = cw[:, :, ::-1]       # reversed taps for backward conv

    out = {
        'hT': np.ascontiguousarray(hid[b].T).astype(bfnp),
        'w_in': np.ascontiguousarray(w_in_cols).astype(bfnp),
        'convw': np.ascontiguousarray(convw),
        'convb': np.ascontiguousarray(
            np.stack([per_dir[d]['cb'].reshape(NPAIR, P) for d in range(2)])
        ).astype(np.float32),
        'w_xp': np.ascontiguousarray(
            np.stack([per_dir[d]['xp'].T for d in range(2)])).astype(bfnp),
        'w_dt': np.ascontiguousarray(
            np.stack([per_dir[d]['dtp'].reshape(NPAIR, P, DTR)
                      .transpose(0, 2, 1) for d in range(2)])).astype(bfnp),
        'dt_bias': np.ascontiguousarray(
            np.stack([per_dir[d]['dtb'].reshape(NPAIR, P) for d in range(2)])
        ).astype(np.float32),
        'Acol': np.ascontiguousarray(
            np.stack([per_dir[d]['A'].reshape(NPAIR, P, DS) for d in range(2)])
        ).astype(np.float32),
        'Dvec': np.ascontiguousarray(
            np.stack([per_dir[d]['Dv'].reshape(NPAIR, P) for d in range(2)])
        ).astype(np.float32),
        'w_out': np.ascontiguousarray(
            w_out_full[:, chans].T.reshape(NPAIR, P, D)).astype(bfnp),
    }
    return out


_module_cache = {}


def _get_module():
    if 'nc' not in _module_cache:
        _module_cache['nc'] = build_module()
    return _module_cache['nc']


def kernel(**inputs):
    nc = _get_module()
    in_maps = [_prep_core_inputs(inputs, c) for c in range(NCORES)]
    res = run_bass_kernel_spmd(nc, in_maps, list(range(NCORES)))
    out = np.zeros((B, L, D), np.float32)
    for c in range(NCORES):
        out[c // 4] += res.results[c]['out_part'].T
    return out
